# revision 1
# baseline (speedup 1.0000x reference)
"""Grouped-GEMM MoE expert MLP kernel for 8 Trainium2 NeuronCores.

Problem: x [8, 2048, 1024] f32, per-group W1 [8, 4096, 1024], b1 [8, 4096],
W2 [8, 1024, 4096], b2 [8, 1024] (torch Linear convention, y = x @ W.T + b):
  h1 = xg @ W1.T + b1        (per group)
  h2 = h1 @ W2.T + b2
Expert-parallel: core i owns group i entirely — no collectives.

Formulation is fully transposed so every DMA is contiguous and biases land on
the partition axis:
  h1T[o, m]   = matmul(lhsT=W1T[h,o] tiles, rhs=xT[h,m] tiles)  + b1[o]
  outT[ho, m] = matmul(lhsT=W2T[o,ho] tiles, rhs=h1T[o,m] tiles) + b2[ho]
(out = lhsT.T @ rhs contracts the partition axis of both operands.)
Host pre-transposes x/W1/W2 per shard and un-transposes the output.

Matmuls run in float32r (full-rate fp32 mode, 1 cycle/row at free dim 512)
with fp32 PSUM accumulation.

Per-core loop structure: 2 m-chunks of 1024 tokens; inside, 8 o-chunks of 512.
GEMM1 for an o-chunk feeds SBUF tiles h1T; GEMM2 accumulates PSUM over an
o-PAIR (1024, 8 k-steps) then folds into an SBUF accumulator (first pair via
ScalarE copy+bias, later pairs via VectorE add) to keep PSUM pressure at
4+4 banks. Weights are streamed per m-chunk (2 x 33.6 MB), x and out once.
"""
import sys

sys.path.insert(0, "/opt/trn_rl_repo")

import numpy as np

import concourse.bass as bass  # noqa: F401  (bass import initializes mybir deps)
import concourse.mybir as mybir
import concourse.tile as tile
from concourse import bacc
from concourse.bass_utils import run_bass_kernel_spmd

NUM_GEMMS = 8
HIDDEN = 1024
INTER = 4096
M = 2048  # tokens per group

M_CHUNK = 1024  # tokens per chunk (2 chunks)
MS = 512        # matmul moving free dim (fp32 max / one PSUM bank)
O_CHUNK = 512   # GEMM1 / weight-DMA granularity along INTER
O_PAIR = 1024   # GEMM2 PSUM accumulation span along INTER (8 k-steps)

f32 = mybir.dt.float32
f32r = mybir.dt.float32r

N_MC = M // M_CHUNK              # 2
N_PAIR = INTER // O_PAIR         # 4
N_MS = M_CHUNK // MS             # 2
KT1 = HIDDEN // 128              # 8 k-tiles for GEMM1
KT2 = O_PAIR // 128              # 8 k-tiles per GEMM2 psum group
N_OT = O_CHUNK // 128            # 4 o-tiles per o-chunk
N_HT = HIDDEN // 128             # 8 hout-tiles

_NC_CACHE = None


def build_nc():
    """Build + compile the single-core program (same on all 8 cores)."""
    global _NC_CACHE
    if _NC_CACHE is not None:
        return _NC_CACHE

    nc = bacc.Bacc("TRN2", target_bir_lowering=False, debug=False, num_devices=8)
    xT = nc.dram_tensor("xT", [HIDDEN, M], f32r, kind="ExternalInput").ap()
    w1T = nc.dram_tensor("w1T", [HIDDEN, INTER], f32r, kind="ExternalInput").ap()
    b1 = nc.dram_tensor("b1", [128, INTER // 128], f32, kind="ExternalInput").ap()
    w2T = nc.dram_tensor("w2T", [INTER, HIDDEN], f32r, kind="ExternalInput").ap()
    b2 = nc.dram_tensor("b2", [128, HIDDEN // 128], f32, kind="ExternalInput").ap()
    outT = nc.dram_tensor("outT", [HIDDEN, M], f32, kind="ExternalOutput").ap()

    ID = mybir.ActivationFunctionType.Identity

    with tile.TileContext(nc) as tc:
        with (
            tc.tile_pool(name="cst", bufs=1) as cst,
            tc.tile_pool(name="xp", bufs=1) as xp,
            tc.tile_pool(name="hp", bufs=1) as hp,
            tc.tile_pool(name="w1p", bufs=3) as w1p,
            tc.tile_pool(name="w2p", bufs=2) as w2p,
            tc.tile_pool(name="h1p", bufs=2) as h1p,
            tc.tile_pool(name="ps1", bufs=4, space="PSUM") as ps1,
            tc.tile_pool(name="ps2", bufs=4, space="PSUM") as ps2,
        ):
            # PE warmup while the first DMAs fill: releases the HAM clock
            # throttle (4/8 -> 8/8, needs ~3.4us of sustained PE activity)
            # before the real matmuls arrive.
            # Warmup matmuls read a framework const tile (loaded in the
            # preamble, before any DMA can land) broadcast along the free
            # dim; plain fp32 runs at 4 cyc/row so a handful of matmuls
            # spans the ~3.4us HAM un-throttle window.
            ps_junk = ps1.tile([128, MS], f32, tag="ps1", name="ps1t")
            cwarm = nc.const_aps.scalar_like(1.0, ps_junk[:, :])
            cbr = cwarm.broadcast_to([128, MS])
            for _ in range(2):
                nc.tensor.matmul(
                    ps_junk[:1, :], cwarm, cbr, start=True, stop=True,
                )

            b1_sb = cst.tile([128, INTER // 128], f32)
            b2_sb = cst.tile([128, HIDDEN // 128], f32)

            for mc in range(N_MC):
                m0 = mc * M_CHUNK
                # x chunk: [HIDDEN, M_CHUNK] -> [128, KT1 * M_CHUNK].
                # Split per k-tile so the first GEMM1 matmuls can start as
                # soon as k-tile 0 lands (subtile deps) instead of after the
                # whole 4.2 MB chunk.
                xt_sb = xp.tile([128, KT1 * M_CHUNK], f32r, tag="xt")
                xt_dma = []
                for k in range(KT1):
                    xt_dma.append((
                        xt_sb[:, k * M_CHUNK:(k + 1) * M_CHUNK],
                        xT[k * 128:(k + 1) * 128, m0:m0 + M_CHUNK],
                    ))
                if mc != 0:
                    # Non-first chunk: one bulk prefetch queued behind
                    # current work.
                    nc.sync.dma_start(
                        xt_sb[:, :].rearrange("p (a m) -> p a m", m=M_CHUNK),
                        xT[:, m0:m0 + M_CHUNK].rearrange(
                            "(a p) m -> p a m", p=128),
                    )
                    xt_dma = []
                # output accumulator: [HIDDEN, M_CHUNK] -> [128, N_HT * M_CHUNK]
                h2_sb = hp.tile([128, N_HT * M_CHUNK], f32, tag="h2")

                for pair in range(N_PAIR):
                    h1_half = []
                    w2_half = []
                    deferred_w2 = []
                    for half in range(2):
                        oc = pair * 2 + half
                        o0 = oc * O_CHUNK
                        cold = mc == 0 and pair == 0 and half == 0
                        # W1T slice [HIDDEN, O_CHUNK] -> [128, KT1 * O_CHUNK]
                        w1_sb = w1p.tile([128, KT1 * O_CHUNK], f32r, tag="w1")
                        if cold:
                            # Cold fill, ordered to match the ms-outer
                            # consumption order of the first GEMM1 pass.
                            # Scalar's queue clears its preamble ~1.5us
                            # before sync's, so the very first k-tile pair
                            # issues there; x loads split per ms-half so
                            # the ms=0 pass isn't gated on ms=1 bytes.
                            def xt_half(k, ms):
                                return (
                                    xt_sb[:, k * M_CHUNK + ms * MS:
                                          k * M_CHUNK + (ms + 1) * MS],
                                    xT[k * 128:(k + 1) * 128,
                                       m0 + ms * MS:m0 + (ms + 1) * MS],
                                )
                            nc.sync.dma_start(
                                w1_sb[:, 0:O_CHUNK],
                                w1T[0:128, o0:o0 + O_CHUNK],
                            )
                            nc.scalar.dma_start(*xt_half(0, 0))
                            nc.scalar.dma_start(b1_sb[:, :], b1[:, :])
                            nc.scalar.dma_start(b2_sb[:, :], b2[:, :])
                            for k in range(1, KT1):
                                nc.sync.dma_start(
                                    w1_sb[:, k * O_CHUNK:(k + 1) * O_CHUNK],
                                    w1T[k * 128:(k + 1) * 128, o0:o0 + O_CHUNK],
                                )
                                nc.sync.dma_start(*xt_half(k, 0))
                            for k in range(KT1):
                                nc.sync.dma_start(*xt_half(k, 1))
                        else:
                            nc.sync.dma_start(
                                w1_sb[:, :].rearrange("p (a o) -> p a o",
                                                      o=O_CHUNK),
                                w1T[:, o0:o0 + O_CHUNK].rearrange(
                                    "(a p) o -> p a o", p=128),
                            )
                        # The cold half's W2 slice queues here, AFTER this
                        # half's W1 — W1(oc1) is needed ~15us before
                        # W2(oc0), and the sync queue delivers in FIFO
                        # order.
                        for args in deferred_w2:
                            nc.sync.dma_start(*args)
                        deferred_w2 = []

                        # GEMM1: h1T[o0:o0+512, m-chunk]
                        h1_sb = h1p.tile([128, N_OT * M_CHUNK], f32r, tag="h1")
                        if cold:
                            # k-outer order: consume k-tiles as they arrive.
                            # ms outer keeps live PSUM groups at N_OT = 4.
                            for ms in range(N_MS):
                                accs = [ps1.tile([128, MS], f32, tag="ps1",
                                                 name="ps1t")
                                        for _ in range(N_OT)]
                                for k in range(KT1):
                                    for ot in range(N_OT):
                                        nc.tensor.matmul(
                                            accs[ot][:, :],
                                            w1_sb[:, k * O_CHUNK + ot * 128:
                                                  k * O_CHUNK + (ot + 1) * 128],
                                            xt_sb[:, k * M_CHUNK + ms * MS:
                                                  k * M_CHUNK + (ms + 1) * MS],
                                            start=(k == 0),
                                            stop=(k == KT1 - 1),
                                        )
                                for ot in range(N_OT):
                                    nc.scalar.activation(
                                        h1_sb[:, ot * M_CHUNK + ms * MS:
                                              ot * M_CHUNK + (ms + 1) * MS],
                                        accs[ot][:, :],
                                        ID,
                                        bias=b1_sb[:, oc * N_OT + ot:
                                                   oc * N_OT + ot + 1],
                                        scale=1.0,
                                    )
                        else:
                            for ot in range(N_OT):
                                accs = [ps1.tile([128, MS], f32, tag="ps1",
                                                 name="ps1t")
                                        for _ in range(N_MS)]
                                for k in range(KT1):
                                    lhsT = w1_sb[:, k * O_CHUNK + ot * 128:
                                                 k * O_CHUNK + (ot + 1) * 128]
                                    for ms in range(N_MS):
                                        nc.tensor.matmul(
                                            accs[ms][:, :],
                                            lhsT,
                                            xt_sb[:, k * M_CHUNK + ms * MS:
                                                  k * M_CHUNK + (ms + 1) * MS],
                                            start=(k == 0),
                                            stop=(k == KT1 - 1),
                                        )
                                for ms in range(N_MS):
                                    nc.scalar.activation(
                                        h1_sb[:, ot * M_CHUNK + ms * MS:
                                              ot * M_CHUNK + (ms + 1) * MS],
                                        accs[ms][:, :],
                                        ID,
                                        bias=b1_sb[:, oc * N_OT + ot:
                                                   oc * N_OT + ot + 1],
                                        scale=1.0,
                                    )
                        h1_half.append(h1_sb)

                        # W2T slice [O_CHUNK, HIDDEN] -> [128, N_OT * HIDDEN].
                        # Emitted after GEMM1 so its DMA queues behind the
                        # critical-path x/W1 loads.
                        w2_sb = w2p.tile([128, N_OT * HIDDEN], f32r, tag="w2")
                        w2_args = (
                            w2_sb[:, :].rearrange("p (a n) -> p a n", n=HIDDEN),
                            w2T[o0:o0 + O_CHUNK, :].rearrange(
                                "(a p) n -> p a n", p=128),
                        )
                        if cold:
                            deferred_w2.append(w2_args)
                        else:
                            nc.sync.dma_start(*w2_args)
                        w2_half.append(w2_sb)

                    # GEMM2 for the o-pair: accumulate 8 k-steps in PSUM,
                    # then fold into h2_sb.
                    for ht in range(N_HT):
                        accs = [ps2.tile([128, MS], f32, tag="ps2", name="ps2t")
                                for _ in range(N_MS)]
                        for k in range(KT2):
                            half, ot = divmod(k, N_OT)
                            lhsT = w2_half[half][:, ot * HIDDEN + ht * 128:
                                                 ot * HIDDEN + (ht + 1) * 128]
                            for ms in range(N_MS):
                                nc.tensor.matmul(
                                    accs[ms][:, :],
                                    lhsT,
                                    h1_half[half][:, ot * M_CHUNK + ms * MS:
                                                  ot * M_CHUNK + (ms + 1) * MS],
                                    start=(k == 0),
                                    stop=(k == KT2 - 1),
                                )
                        for ms in range(N_MS):
                            dst = h2_sb[:, ht * M_CHUNK + ms * MS:
                                        ht * M_CHUNK + (ms + 1) * MS]
                            if pair == 0:
                                nc.scalar.activation(
                                    dst, accs[ms][:, :], ID,
                                    bias=b2_sb[:, ht:ht + 1], scale=1.0,
                                )
                            else:
                                nc.vector.tensor_add(dst, dst, accs[ms][:, :])
                        if pair == N_PAIR - 1:
                            # Stream each hout-row-block out as soon as its
                            # last fold lands — keeps the kernel tail short.
                            # One DMA per block: each dma_start pays ~2us of
                            # completion latency, so fewer, larger transfers
                            # win at the very end.
                            nc.sync.dma_start(
                                outT[ht * 128:(ht + 1) * 128,
                                     m0:m0 + M_CHUNK],
                                h2_sb[:, ht * M_CHUNK:(ht + 1) * M_CHUNK],
                            )

    nc.compile()
    _NC_CACHE = nc
    return nc


def _prep_core_inputs(x, W1, b1, W2, b2, i):
    return {
        "xT": np.ascontiguousarray(np.asarray(x[i], dtype=np.float32).T),
        "w1T": np.ascontiguousarray(np.asarray(W1[i], dtype=np.float32).T),
        "b1": np.ascontiguousarray(
            np.asarray(b1[i], dtype=np.float32).reshape(INTER // 128, 128).T),
        "w2T": np.ascontiguousarray(np.asarray(W2[i], dtype=np.float32).T),
        "b2": np.ascontiguousarray(
            np.asarray(b2[i], dtype=np.float32).reshape(HIDDEN // 128, 128).T),
    }


def kernel(x, W1, b1, W2, b2, _trace=False, _trace_kwargs=None):
    x = np.asarray(x, dtype=np.float32)
    orig_shape = x.shape
    xg = x.reshape(NUM_GEMMS, M, HIDDEN)

    nc = build_nc()
    in_maps = [_prep_core_inputs(xg, W1, b1, W2, b2, i) for i in range(NUM_GEMMS)]
    res = None
    for attempt in range(3):
        try:
            res = run_bass_kernel_spmd(
                nc, in_maps, list(range(NUM_GEMMS)),
                trace=_trace, **(_trace_kwargs or {}),
            )
            break
        except Exception:
            # transient NRT_EXEC_UNIT_UNRECOVERABLE has been observed on
            # rapid repeated runs; a short pause and retry recovers
            if attempt == 2:
                raise
            import time
            time.sleep(20)
    out = np.stack(
        [res.results[i]["outT"].T for i in range(NUM_GEMMS)], axis=0
    ).reshape(orig_shape).astype(np.float32)
    if _trace:
        return out, res
    return out



# revision 6
# speedup vs baseline: 1.2617x; 1.2617x over previous
"""Grouped-GEMM MoE expert MLP kernel for 8 Trainium2 NeuronCores.

Problem: x [8, 2048, 1024] f32, per-group W1 [8, 4096, 1024], b1 [8, 4096],
W2 [8, 1024, 4096], b2 [8, 1024] (torch Linear convention, y = x @ W.T + b):
  h1 = xg @ W1.T + b1        (per group)
  h2 = h1 @ W2.T + b2
Expert-parallel: core i owns group i entirely — no collectives.

Formulation is fully transposed so every DMA is contiguous and biases land on
the partition axis:
  h1T[o, m]   = matmul(lhsT=W1T[h,o] tiles, rhs=xT[h,m] tiles)  + b1[o]
  outT[ho, m] = matmul(lhsT=W2T[o,ho] tiles, rhs=h1T[o,m] tiles) + b2[ho]
(out = lhsT.T @ rhs contracts the partition axis of both operands.)
Host pre-transposes x/W1/W2 per shard and un-transposes the output.

Matmuls run in bfloat16 with fp32 PSUM accumulation. f32r (full-rate fp32)
measured 272 ns per 512-row matmul: the PE's SBUF read path (512 B/cycle)
serves both the moving rows (512x512B) and the next 128x128 stationary load
(64 KB f32) -> 640 cycles, SBUF-bound. bf16 halves both streams (160 KB ->
320 cycles < 512 compute cycles), so the weight loads hide entirely and each
matmul runs at the 1 cycle/row compute floor. Accuracy: bf16 inputs with f32
accumulation give ~3e-3 global rel err on this problem (gate 2e-2).

Per-core loop structure: 2 m-chunks of 1024 tokens; inside, 8 o-chunks of 512.
GEMM1 for an o-chunk feeds SBUF tiles h1T; GEMM2 accumulates PSUM over an
o-PAIR (1024, 8 k-steps) then folds into an SBUF accumulator (first pair via
ScalarE copy+bias, later pairs via VectorE add) to keep PSUM pressure at
4+4 banks. Weights are streamed per m-chunk (2 x 33.6 MB), x and out once.
"""
import sys

sys.path.insert(0, "/opt/trn_rl_repo")

import ml_dtypes
import numpy as np

import concourse.bass as bass  # noqa: F401  (bass import initializes mybir deps)
import concourse.mybir as mybir
import concourse.tile as tile
from concourse import bacc
from concourse.bass_utils import run_bass_kernel_spmd

NUM_GEMMS = 8
HIDDEN = 1024
INTER = 4096
M = 2048  # tokens per group

M_CHUNK = 1024  # tokens per chunk (2 chunks)
MS = 512        # matmul moving free dim (fp32 max / one PSUM bank)
O_CHUNK = 512   # GEMM1 / weight-DMA granularity along INTER
O_PAIR = 1024   # GEMM2 PSUM accumulation span along INTER (8 k-steps)

f32 = mybir.dt.float32
f32r = mybir.dt.float32r
bf16 = mybir.dt.bfloat16

N_MC = M // M_CHUNK              # 2
N_PAIR = INTER // O_PAIR         # 4
N_MS = M_CHUNK // MS             # 2
KT1 = HIDDEN // 128              # 8 k-tiles for GEMM1
KT2 = O_PAIR // 128              # 8 k-tiles per GEMM2 psum group
N_OT = O_CHUNK // 128            # 4 o-tiles per o-chunk
N_HT = HIDDEN // 128             # 8 hout-tiles

_NC_CACHE = None


def build_nc():
    """Build + compile the single-core program (same on all 8 cores)."""
    global _NC_CACHE
    if _NC_CACHE is not None:
        return _NC_CACHE

    nc = bacc.Bacc("TRN2", target_bir_lowering=False, debug=False, num_devices=8)
    xT = nc.dram_tensor("xT", [HIDDEN, M], bf16, kind="ExternalInput").ap()
    w1T = nc.dram_tensor("w1T", [HIDDEN, INTER], bf16, kind="ExternalInput").ap()
    b1 = nc.dram_tensor("b1", [128, INTER // 128], f32, kind="ExternalInput").ap()
    w2T = nc.dram_tensor("w2T", [INTER, HIDDEN], bf16, kind="ExternalInput").ap()
    b2 = nc.dram_tensor("b2", [128, HIDDEN // 128], f32, kind="ExternalInput").ap()
    outT = nc.dram_tensor("outT", [HIDDEN, M], f32, kind="ExternalOutput").ap()

    ID = mybir.ActivationFunctionType.Identity

    with tile.TileContext(nc) as tc:
        with (
            tc.tile_pool(name="cst", bufs=1) as cst,
            tc.tile_pool(name="xp", bufs=1) as xp,
            tc.tile_pool(name="hp", bufs=1) as hp,
            tc.tile_pool(name="w1p", bufs=3) as w1p,
            tc.tile_pool(name="w2p", bufs=2) as w2p,
            tc.tile_pool(name="h1p", bufs=2) as h1p,
            tc.tile_pool(name="ps1", bufs=4, space="PSUM") as ps1,
            tc.tile_pool(name="ps2", bufs=4, space="PSUM") as ps2,
        ):
            # PE warmup while the first DMAs fill: releases the HAM clock
            # throttle (4/8 -> 8/8, needs ~3.4us of sustained PE activity)
            # before the real matmuls arrive.
            # Warmup matmuls read a framework const tile (loaded in the
            # preamble, before any DMA can land) broadcast along the free
            # dim; plain fp32 runs at 4 cyc/row so a handful of matmuls
            # spans the ~3.4us HAM un-throttle window.
            ps_junk = ps1.tile([128, MS], f32, tag="ps1", name="ps1t")
            cwarm = nc.const_aps.scalar_like(1.0, ps_junk[:, :])
            cbr = cwarm.broadcast_to([128, MS])
            for _ in range(2):
                nc.tensor.matmul(
                    ps_junk[:1, :], cwarm, cbr, start=True, stop=True,
                )

            b1_sb = cst.tile([128, INTER // 128], f32)
            b2_sb = cst.tile([128, HIDDEN // 128], f32)

            for mc in range(N_MC):
                m0 = mc * M_CHUNK
                # x chunk: [HIDDEN, M_CHUNK] -> [128, KT1 * M_CHUNK].
                # Split per k-tile so the first GEMM1 matmuls can start as
                # soon as k-tile 0 lands (subtile deps) instead of after the
                # whole 4.2 MB chunk.
                xt_sb = xp.tile([128, KT1 * M_CHUNK], bf16, tag="xt")
                xt_dma = []
                for k in range(KT1):
                    xt_dma.append((
                        xt_sb[:, k * M_CHUNK:(k + 1) * M_CHUNK],
                        xT[k * 128:(k + 1) * 128, m0:m0 + M_CHUNK],
                    ))
                if mc != 0:
                    # Non-first chunk: one bulk prefetch queued behind
                    # current work.
                    nc.sync.dma_start(
                        xt_sb[:, :].rearrange("p (a m) -> p a m", m=M_CHUNK),
                        xT[:, m0:m0 + M_CHUNK].rearrange(
                            "(a p) m -> p a m", p=128),
                    )
                    xt_dma = []
                # output accumulator: [HIDDEN, M_CHUNK] -> [128, N_HT * M_CHUNK]
                h2_sb = hp.tile([128, N_HT * M_CHUNK], f32, tag="h2")

                for pair in range(N_PAIR):
                    h1_half = []
                    w2_half = []
                    deferred_w2 = []
                    for half in range(2):
                        oc = pair * 2 + half
                        o0 = oc * O_CHUNK
                        cold = mc == 0 and pair == 0 and half == 0
                        # W1T slice [HIDDEN, O_CHUNK] -> [128, KT1 * O_CHUNK]
                        w1_sb = w1p.tile([128, KT1 * O_CHUNK], bf16, tag="w1")
                        if cold:
                            # Cold fill, ordered to match the ms-outer
                            # consumption order of the first GEMM1 pass.
                            # Scalar's queue clears its preamble ~1.5us
                            # before sync's, so the very first k-tile pair
                            # issues there; x loads split per ms-half so
                            # the ms=0 pass isn't gated on ms=1 bytes.
                            def xt_half(k, ms):
                                return (
                                    xt_sb[:, k * M_CHUNK + ms * MS:
                                          k * M_CHUNK + (ms + 1) * MS],
                                    xT[k * 128:(k + 1) * 128,
                                       m0 + ms * MS:m0 + (ms + 1) * MS],
                                )
                            nc.sync.dma_start(
                                w1_sb[:, 0:O_CHUNK],
                                w1T[0:128, o0:o0 + O_CHUNK],
                            )
                            nc.scalar.dma_start(*xt_half(0, 0))
                            nc.scalar.dma_start(b1_sb[:, :], b1[:, :])
                            nc.scalar.dma_start(b2_sb[:, :], b2[:, :])
                            for k in range(1, KT1):
                                nc.sync.dma_start(
                                    w1_sb[:, k * O_CHUNK:(k + 1) * O_CHUNK],
                                    w1T[k * 128:(k + 1) * 128, o0:o0 + O_CHUNK],
                                )
                                nc.sync.dma_start(*xt_half(k, 0))
                            for k in range(KT1):
                                nc.sync.dma_start(*xt_half(k, 1))
                        else:
                            nc.sync.dma_start(
                                w1_sb[:, :].rearrange("p (a o) -> p a o",
                                                      o=O_CHUNK),
                                w1T[:, o0:o0 + O_CHUNK].rearrange(
                                    "(a p) o -> p a o", p=128),
                            )
                        # The cold half's W2 slice queues here, AFTER this
                        # half's W1 — W1(oc1) is needed ~15us before
                        # W2(oc0), and the sync queue delivers in FIFO
                        # order.
                        for args in deferred_w2:
                            nc.sync.dma_start(*args)
                        deferred_w2 = []

                        # GEMM1: h1T[o0:o0+512, m-chunk]
                        h1_sb = h1p.tile([128, N_OT * M_CHUNK], bf16, tag="h1")
                        if cold:
                            # k-outer order: consume k-tiles as they arrive.
                            # ms outer keeps live PSUM groups at N_OT = 4.
                            for ms in range(N_MS):
                                accs = [ps1.tile([128, MS], f32, tag="ps1",
                                                 name="ps1t")
                                        for _ in range(N_OT)]
                                for k in range(KT1):
                                    for ot in range(N_OT):
                                        nc.tensor.matmul(
                                            accs[ot][:, :],
                                            w1_sb[:, k * O_CHUNK + ot * 128:
                                                  k * O_CHUNK + (ot + 1) * 128],
                                            xt_sb[:, k * M_CHUNK + ms * MS:
                                                  k * M_CHUNK + (ms + 1) * MS],
                                            start=(k == 0),
                                            stop=(k == KT1 - 1),
                                        )
                                for ot in range(N_OT):
                                    nc.scalar.activation(
                                        h1_sb[:, ot * M_CHUNK + ms * MS:
                                              ot * M_CHUNK + (ms + 1) * MS],
                                        accs[ot][:, :],
                                        ID,
                                        bias=b1_sb[:, oc * N_OT + ot:
                                                   oc * N_OT + ot + 1],
                                        scale=1.0,
                                    )
                        else:
                            for ot in range(N_OT):
                                accs = [ps1.tile([128, MS], f32, tag="ps1",
                                                 name="ps1t")
                                        for _ in range(N_MS)]
                                for k in range(KT1):
                                    lhsT = w1_sb[:, k * O_CHUNK + ot * 128:
                                                 k * O_CHUNK + (ot + 1) * 128]
                                    for ms in range(N_MS):
                                        nc.tensor.matmul(
                                            accs[ms][:, :],
                                            lhsT,
                                            xt_sb[:, k * M_CHUNK + ms * MS:
                                                  k * M_CHUNK + (ms + 1) * MS],
                                            start=(k == 0),
                                            stop=(k == KT1 - 1),
                                        )
                                for ms in range(N_MS):
                                    nc.scalar.activation(
                                        h1_sb[:, ot * M_CHUNK + ms * MS:
                                              ot * M_CHUNK + (ms + 1) * MS],
                                        accs[ms][:, :],
                                        ID,
                                        bias=b1_sb[:, oc * N_OT + ot:
                                                   oc * N_OT + ot + 1],
                                        scale=1.0,
                                    )
                        h1_half.append(h1_sb)

                        # W2T slice [O_CHUNK, HIDDEN] -> [128, N_OT * HIDDEN].
                        # Emitted after GEMM1 so its DMA queues behind the
                        # critical-path x/W1 loads.
                        w2_sb = w2p.tile([128, N_OT * HIDDEN], bf16, tag="w2")
                        w2_args = (
                            w2_sb[:, :].rearrange("p (a n) -> p a n", n=HIDDEN),
                            w2T[o0:o0 + O_CHUNK, :].rearrange(
                                "(a p) n -> p a n", p=128),
                        )
                        if cold:
                            deferred_w2.append(w2_args)
                        else:
                            nc.sync.dma_start(*w2_args)
                        w2_half.append(w2_sb)

                    # GEMM2 for the o-pair: accumulate 8 k-steps in PSUM,
                    # then fold into h2_sb.
                    for ht in range(N_HT):
                        accs = [ps2.tile([128, MS], f32, tag="ps2", name="ps2t")
                                for _ in range(N_MS)]
                        for k in range(KT2):
                            half, ot = divmod(k, N_OT)
                            lhsT = w2_half[half][:, ot * HIDDEN + ht * 128:
                                                 ot * HIDDEN + (ht + 1) * 128]
                            for ms in range(N_MS):
                                nc.tensor.matmul(
                                    accs[ms][:, :],
                                    lhsT,
                                    h1_half[half][:, ot * M_CHUNK + ms * MS:
                                                  ot * M_CHUNK + (ms + 1) * MS],
                                    start=(k == 0),
                                    stop=(k == KT2 - 1),
                                )
                        for ms in range(N_MS):
                            dst = h2_sb[:, ht * M_CHUNK + ms * MS:
                                        ht * M_CHUNK + (ms + 1) * MS]
                            if pair == 0:
                                nc.scalar.activation(
                                    dst, accs[ms][:, :], ID,
                                    bias=b2_sb[:, ht:ht + 1], scale=1.0,
                                )
                            else:
                                nc.vector.tensor_add(dst, dst, accs[ms][:, :])
                        if pair == N_PAIR - 1:
                            # Stream each hout-row-block out as soon as its
                            # last fold lands — keeps the kernel tail short.
                            # One DMA per block: each dma_start pays ~2us of
                            # completion latency, so fewer, larger transfers
                            # win at the very end.
                            nc.sync.dma_start(
                                outT[ht * 128:(ht + 1) * 128,
                                     m0:m0 + M_CHUNK],
                                h2_sb[:, ht * M_CHUNK:(ht + 1) * M_CHUNK],
                            )

    nc.compile()
    _NC_CACHE = nc
    return nc


def _prep_core_inputs(x, W1, b1, W2, b2, i):
    bf = ml_dtypes.bfloat16
    return {
        "xT": np.ascontiguousarray(np.asarray(x[i], dtype=np.float32).T).astype(bf),
        "w1T": np.ascontiguousarray(
            np.asarray(W1[i], dtype=np.float32).T).astype(bf),
        "b1": np.ascontiguousarray(
            np.asarray(b1[i], dtype=np.float32).reshape(INTER // 128, 128).T),
        "w2T": np.ascontiguousarray(
            np.asarray(W2[i], dtype=np.float32).T).astype(bf),
        "b2": np.ascontiguousarray(
            np.asarray(b2[i], dtype=np.float32).reshape(HIDDEN // 128, 128).T),
    }


def kernel(x, W1, b1, W2, b2, _trace=False, _trace_kwargs=None):
    x = np.asarray(x, dtype=np.float32)
    orig_shape = x.shape
    xg = x.reshape(NUM_GEMMS, M, HIDDEN)

    nc = build_nc()
    in_maps = [_prep_core_inputs(xg, W1, b1, W2, b2, i) for i in range(NUM_GEMMS)]
    res = None
    for attempt in range(3):
        try:
            res = run_bass_kernel_spmd(
                nc, in_maps, list(range(NUM_GEMMS)),
                trace=_trace, **(_trace_kwargs or {}),
            )
            break
        except Exception:
            # transient NRT_EXEC_UNIT_UNRECOVERABLE has been observed on
            # rapid repeated runs; a short pause and retry recovers
            if attempt == 2:
                raise
            import time
            time.sleep(20)
    out = np.stack(
        [res.results[i]["outT"].T for i in range(NUM_GEMMS)], axis=0
    ).reshape(orig_shape).astype(np.float32)
    if _trace:
        return out, res
    return out

